# revision 34
# baseline (speedup 1.0000x reference)
"""BigBird block-sparse attention on 8 Trainium2 NeuronCores.

Problem: B=2, H=16, S=4096, D=64, block 64, 3 random blocks/row, masks all-ones.
Sharding: the 32 (b,h) pairs are data-parallel; each of the 8 cores handles 4.

Per (b,h), query block-rows are processed in PAIRS (128 queries):
  - full pair   = rows (0, 63): full attention over all 4096 keys
  - special pair= rows (1, 62): blocks {0,1,2,63}+rand / {0,61,62,63}+rand
  - middle pairs l=0..29 = rows (2l+2, 2l+3): window + globals + 3 rand each

For each pair, scores are computed TRANSPOSED: S^T[k, q] = K @ Q^T via
matmuls with kT chunks (128 keys) as the stationary operand.  exp() on the
scalar engine then writes P^T straight to SBUF (no on-chip transpose of P
needed).  PV accumulates ctx^T[d, q] with V chunks as weights; an appended
ones-column in the V staging makes row 64 of ctx^T the softmax denominator.
One small PE transpose per pair brings ctx back to natural [q, d] layout.
Random-block PV contributions are then accumulated directly onto that
natural-layout PSUM using the exp'd score chunks as weights and
dynamically-indexed slices of a duplicated V table as the moving operand
(so no V gather is ever materialized).  A reciprocal-scaled copy finally
normalizes the context.

Softmax skips the max-subtraction: inputs are N(0,1) so scores/8 ~ N(0,1);
exp stays well inside fp32 range and matches the reference analytically.

Random K^T columns are gathered on the otherwise-idle GPSIMD engine with
one batched ap_gather per (b,h) (SPMD-safe: indices are per-core data).
"""

import dataclasses

import numpy as np

import concourse.bass as bass
import concourse.bacc as bacc
import concourse.mybir as mybir
import concourse.tile as tile
from concourse.masks import make_identity

B, H, S, D = 2, 16, 4096, 64
BS = 64
NBLK = S // BS            # 64
NCORES = 8
NBH = (B * H) // NCORES   # 4 (b,h) pairs per core
NMID = 30                 # middle pairs per (b,h)
NRP = NMID + 1            # rand-carrying pairs: special + 30 middle
NG = 192                  # gathered rand K blocks per (b,h), padded from 186
F16 = mybir.dt.float16
F32 = mybir.dt.float32
I16 = mybir.dt.int16
I32 = mybir.dt.int32
PE = mybir.EngineType.PE
NEG = -30000.0            # exp(NEG/8) == 0.0 exactly in fp32


def _rows2(ap2, r1, r2, w):
    """[R, C] (DRAM) AP -> [2, w, C] view of row ranges {r1:r1+w, r2:r2+w}."""
    b = ap2[r1:r1 + w, :]
    row_step = b.ap[0][0]
    return dataclasses.replace(
        b, ap=[[(r2 - r1) * row_step, 2], list(b.ap[0]), list(b.ap[1])]
    )


def _pair_descs():
    """Chunk/mask descriptors for the 32 query-row pairs of one (b,h).

    chunk k-spec: ("kt", col)  contiguous 128 cols of kT
                  ("ktg",) / ("ktg2",)  global stagings
                  ("ktr", j)  rand gather slots (2j, 2j+1)
    chunk v-spec: ("shift", m) | ("g",) | ("m0",) | ("m1",)   (rand: via vdup)
    mask: (pslice, chunk_idx, cslice) -> memset on S^T psum
    """
    pairs = []
    # --- full pair: rows 0 & 63 ---
    chunks = [(("ktg",), ("g",))]
    chunks += [(("kt", 64 + 128 * m), ("shift", m)) for m in range(31)]
    pairs.append(dict(
        name="full", q=("two", 0, 4032), out=("two", 0, 4032),
        chunks=chunks, masks=[], rand=None,
    ))
    # --- special pair: rows 1 & 62 ---
    chunks = [
        (("kt", 0), ("m0",)),            # blocks (0, 1)
        (("ktg2",), ("m1",)),            # blocks (2, 63)
        (("kt", 3904), ("shift", 30)),   # blocks (61, 62)
        (("ktr", 0), ("vr", 0)),
        (("ktr", 1), ("vr", 1)),
        (("ktr", 2), ("vr", 2)),
    ]
    masks = [
        (slice(64, 128), 0, slice(64, 128)),  # blk1 invalid for row62
        (slice(0, 64), 1, slice(64, 128)),    # blk2 invalid for row62
        (slice(0, 128), 2, slice(0, 64)),     # blks 61,62 invalid for row1
        (slice(0, 128), 3, slice(64, 128)),   # randA invalid for row62
        (slice(0, 64), 4, slice(64, 128)),    # rA2 invalid for row62
        (slice(64, 128), 4, slice(0, 64)),    # rB0 invalid for row1
        (slice(0, 128), 5, slice(0, 64)),     # randB invalid for row1
    ]
    pairs.append(dict(
        name="special", q=("two", 64, 3968), out=("two", 64, 3968),
        chunks=chunks, masks=masks, rand=0,
    ))
    # --- middle pairs ---
    for l in range(NMID):
        chunks = [
            (("kt", 64 * (2 * l + 1)), ("shift", l)),      # blocks (2l+1, 2l+2)
            (("kt", 64 * (2 * l + 3)), ("shift", l + 1)),  # blocks (2l+3, 2l+4)
            (("ktg",), ("g",)),                            # blocks (0, 63)
            (("ktr", 0), ("vr", 0)),
            (("ktr", 1), ("vr", 1)),
            (("ktr", 2), ("vr", 2)),
        ]
        masks = [
            (slice(0, 64), 0, slice(64, 128)),    # blk 2l+1 invalid for row B
            (slice(64, 128), 1, slice(0, 64)),    # blk 2l+4 invalid for row A
            (slice(0, 128), 3, slice(64, 128)),   # randA invalid for B
            (slice(0, 64), 4, slice(64, 128)),    # rA2 invalid for B
            (slice(64, 128), 4, slice(0, 64)),    # rB0 invalid for A
            (slice(0, 128), 5, slice(0, 64)),     # randB invalid for A
        ]
        pairs.append(dict(
            name=f"mid{l}", q=("one", 128 * (l + 1)), out=("one", 128 * (l + 1)),
            chunks=chunks, masks=masks, rand=l + 1,
        ))
    return pairs


PAIRS = _pair_descs()


def build_program():
    nc = bacc.Bacc(
        "TRN2",
        target_bir_lowering=False,
        debug=False,
        num_devices=NCORES,
    )
    qT_d = nc.dram_tensor("qT", [NBH, D, S], F16, kind="ExternalInput")
    kT_d = nc.dram_tensor("kT", [NBH, D, S], F16, kind="ExternalInput")
    # one zero pad row per (b,h): the random-V DMA reads 65-wide overlapping
    # rows (the 65th column is replaced by the ones-memset afterwards)
    v_d = nc.dram_tensor("v", [NBH, S + 1, D], F16, kind="ExternalInput")
    idxg_d = nc.dram_tensor("idxg", [NBH, D, 12], I16, kind="ExternalInput")
    rnd_d = nc.dram_tensor("rnd", [NBH, 1, NG], I32, kind="ExternalInput")
    out_d = nc.dram_tensor("out", [NBH, S, D], F32, kind="ExternalOutput")

    with tile.TileContext(nc) as tc:
        with (
            tc.tile_pool(name="const", bufs=1) as cpool,
            tc.tile_pool(name="bh", bufs=2) as bhpool,
            tc.tile_pool(name="work", bufs=3) as wpool,
            tc.tile_pool(name="fin", bufs=3) as fpool,
            tc.tile_pool(name="ps_s", bufs=2, space="PSUM") as ps_s,
            tc.tile_pool(name="ps_ctxT", bufs=2, space="PSUM") as ps_ctxT,
            tc.tile_pool(name="ps_ctx2", bufs=2, space="PSUM") as ps_ctx2,
        ):
            ident = cpool.tile([65, 65], F32)
            make_identity(nc, ident[:])

            for bh in range(NBH):
                qT_sb = bhpool.tile([D, S], F16, tag="qT")
                nc.sync.dma_start(out=qT_sb[:], in_=qT_d[bh])
                kT_sb = bhpool.tile([D, S], F16, tag="kT")
                nc.sync.dma_start(out=kT_sb[:], in_=kT_d[bh])

                # V stagings, each with a ones-column at slot 64
                vshift = bhpool.tile([128, 31, 65], F16, tag="vshift")
                nc.sync.dma_start(
                    out=vshift[:, :, 0:64],
                    in_=v_d[bh][64:4032, :].rearrange("(m p) d -> p m d", p=128),
                )
                nc.vector.memset(vshift[:, :, 64:65], 1.0)
                vg = bhpool.tile([128, 65], F16, tag="vg")
                nc.sync.dma_start(out=vg[:, 0:64], in_=_rows2(v_d[bh], 0, 4032, 64))
                nc.vector.memset(vg[:, 64:65], 1.0)
                vm0 = bhpool.tile([128, 65], F16, tag="vm0")
                nc.sync.dma_start(out=vm0[:, 0:64], in_=v_d[bh][0:128, :])
                nc.vector.memset(vm0[:, 64:65], 1.0)
                vm1 = bhpool.tile([128, 65], F16, tag="vm1")
                nc.sync.dma_start(out=vm1[:, 0:64], in_=_rows2(v_d[bh], 128, 4032, 64))
                nc.vector.memset(vm1[:, 64:65], 1.0)

                # 65-wide overlapping-rows view of V for the random gathers
                vsrc65 = dataclasses.replace(
                    v_d[bh][0:S, :], ap=[[64, S], [1, 65]])

                # batched K^T random-column gather on GPSIMD
                idxg_sb = bhpool.tile([D, 12], I16, tag="idxg")
                nc.sync.dma_start(out=idxg_sb[:], in_=idxg_d[bh])
                rnd_sb = bhpool.tile([1, NG], I32, tag="rnd")
                nc.sync.dma_start(out=rnd_sb[:], in_=rnd_d[bh])
                ktr_all = bhpool.tile([D, NG * BS], F16, tag="ktr_all")
                nc.gpsimd.ap_gather(
                    out_ap=ktr_all[:],
                    in_ap=kT_sb[:],
                    idxs_ap=idxg_sb[:],
                    channels=D,
                    num_elems=NBLK,
                    d=BS,
                    num_idxs=NG,
                )

                # contiguous stagings for non-contiguous matmul operands
                ktg = bhpool.tile([D, 128], F16, tag="ktg")  # blocks (0, 63)
                nc.vector.tensor_copy(ktg[:, 0:64], kT_sb[:, 0:64])
                nc.vector.tensor_copy(ktg[:, 64:128], kT_sb[:, 4032:4096])
                ktg2 = bhpool.tile([D, 128], F16, tag="ktg2")  # blocks (2, 63)
                nc.vector.tensor_copy(ktg2[:, 0:64], kT_sb[:, 128:192])
                nc.vector.tensor_copy(ktg2[:, 64:128], kT_sb[:, 4032:4096])

                for pair in PAIRS:
                    rp = pair["rand"]
                    vr = None
                    if rp is not None:
                        # random V blocks: 6 register-offset DMAs from DRAM
                        vr = wpool.tile([128, 3, 65], F16, tag="vr")
                        for s in range(6):
                            val = nc.values_load(
                                rnd_sb[0:1, 6 * rp + s:6 * rp + s + 1],
                                engines=(mybir.EngineType.SP,),
                            )
                            h0 = 64 * (s % 2)
                            nc.sync.dma_start(
                                out=vr[h0:h0 + 64, s // 2, :],
                                in_=vsrc65[bass.ts(val, 64), :],
                            )
                        nc.vector.memset(vr[:, :, 64:65], 1.0)

                    def kap(spec):
                        if spec[0] == "kt":
                            return kT_sb[:, spec[1]:spec[1] + 128]
                        if spec[0] == "ktg":
                            return ktg[:]
                        if spec[0] == "ktg2":
                            return ktg2[:]
                        assert spec[0] == "ktr"
                        base = BS * (6 * rp + 2 * spec[1])
                        return ktr_all[:, base:base + 128]

                    def vap(spec):
                        if spec[0] == "shift":
                            return vshift[:, spec[1], :]
                        if spec[0] == "g":
                            return vg[:]
                        if spec[0] == "m0":
                            return vm0[:]
                        if spec[0] == "m1":
                            return vm1[:]
                        assert spec[0] == "vr"
                        return vr[:, spec[1], :]

                    if pair["q"][0] == "one":
                        q_ap = qT_sb[:, pair["q"][1]:pair["q"][1] + 128]
                    else:
                        o1, o2 = pair["q"][1], pair["q"][2]
                        qts = wpool.tile([D, 128], F16, tag="qts")
                        nc.vector.tensor_copy(qts[:, 0:64], qT_sb[:, o1:o1 + 64])
                        nc.vector.tensor_copy(qts[:, 64:128], qT_sb[:, o2:o2 + 64])
                        q_ap = qts[:]

                    chunks = pair["chunks"]
                    n_ctxT = len(chunks)
                    ctxT = ps_ctxT.tile([65, 128], F32, tag="ctxT")
                    ci = 0
                    for g0 in range(0, len(chunks), 6):
                        gch = chunks[g0:g0 + 6]
                        ng = len(gch)
                        s_ps = ps_s.tile([128, 6, 128], F32, tag="s")
                        pt = wpool.tile([128, 6, 128], F16, tag="pt")
                        for j, (ks, _) in enumerate(gch):
                            nc.tensor.matmul(
                                out=s_ps[:, j, :], lhsT=kap(ks), rhs=q_ap,
                                start=True, stop=True,
                            )
                        if g0 == 0:
                            for (psl, cj, csl) in pair["masks"]:
                                nc.vector.memset(s_ps[psl, cj, csl], NEG)
                        nc.scalar.activation(
                            out=pt[:, 0:ng, :], in_=s_ps[:, 0:ng, :],
                            func=mybir.ActivationFunctionType.Exp, scale=0.125,
                        )
                        for j, (_, vs) in enumerate(gch):
                            nc.tensor.matmul(
                                out=ctxT[:], lhsT=vap(vs), rhs=pt[:, j, :],
                                start=(ci == 0), stop=(ci == n_ctxT - 1),
                                skip_group_check=True,
                            )
                            ci += 1

                    ctxT_sb = fpool.tile([65, 128], F32, tag="ctxT_sb")
                    nc.scalar.copy(ctxT_sb[:], ctxT[:])
                    ctx2 = ps_ctx2.tile([128, 65], F32, tag="ctx2")
                    nc.tensor.transpose(ctx2[:], ctxT_sb[:], ident[:])
                    recip = fpool.tile([128, 1], F32, tag="recip")
                    nc.vector.reciprocal(recip[:], ctx2[:, 64:65])
                    outsb = fpool.tile([128, 64], F32, tag="outsb")
                    nc.scalar.activation(
                        out=outsb[:], in_=ctx2[:, 0:64],
                        func=mybir.ActivationFunctionType.Copy,
                        scale=recip[:, 0:1],
                    )
                    if pair["out"][0] == "one":
                        o_ap = out_d[bh][pair["out"][1]:pair["out"][1] + 128, :]
                    else:
                        o_ap = _rows2(out_d[bh], pair["out"][1], pair["out"][2], 64)
                    nc.sync.dma_start(out=o_ap, in_=outsb[:])

    nc.compile()
    return nc


_PROGRAM = None


def _get_program():
    global _PROGRAM
    if _PROGRAM is None:
        _PROGRAM = build_program()
    return _PROGRAM


def make_core_inputs(q, k, v, rand, bh_slice):
    """Build one core's input map from full [32, S, D] arrays (fp32)."""
    qs = q[bh_slice]
    ks = k[bh_slice]
    vs = v[bh_slice]
    rs = rand[bh_slice]  # [NBH, 62, 3]
    qT = np.ascontiguousarray(qs.transpose(0, 2, 1)).astype(np.float16)
    kT = np.ascontiguousarray(ks.transpose(0, 2, 1)).astype(np.float16)
    vv = np.zeros((NBH, S + 1, D), np.float16)
    vv[:, :S, :] = vs.astype(np.float16)

    rnd = np.zeros((NBH, 1, NG), np.int32)
    for n in range(NBH):
        for i in range(NRP):
            if i == 0:
                ra, rb = rs[n, 0], rs[n, 61]        # rows 1 and 62
            else:
                l = i - 1
                ra, rb = rs[n, 2 * l + 1], rs[n, 2 * l + 2]
            rnd[n, 0, 6 * i:6 * i + 6] = np.concatenate([ra, rb])
    # ap_gather wrapped index layout: index i lives at partition 16g + i%16,
    # free slot i//16, replicated across the 4 groups of 16 partitions
    idxg = np.empty((NBH, D, 12), np.int16)
    for n in range(NBH):
        wrapped = rnd[n, 0].astype(np.int16).reshape(12, 16).T  # [16, 12]
        idxg[n] = np.tile(wrapped, (4, 1))
    return {"qT": qT, "kT": kT, "v": vv, "idxg": idxg, "rnd": rnd}


def kernel(query, key, value, from_blocked_mask=None, to_blocked_mask=None,
           rand_attn=None, **_ignored):
    # masks are all-ones in this problem's input distribution; the block
    # structure (window/global/random) is handled explicitly.
    q = np.asarray(query, np.float32).reshape(B * H, S, D)
    k = np.asarray(key, np.float32).reshape(B * H, S, D)
    v = np.asarray(value, np.float32).reshape(B * H, S, D)
    rand = np.asarray(rand_attn).reshape(B * H, NBLK - 2, 3).astype(np.int32)

    in_maps = [
        make_core_inputs(q, k, v, rand, slice(c * NBH, (c + 1) * NBH))
        for c in range(NCORES)
    ]
    nc = _get_program()
    from concourse import bass_utils
    res = bass_utils.run_bass_kernel_spmd(nc, in_maps, core_ids=list(range(NCORES)))
    out = np.stack([r["out"] for r in res.results])  # [8, NBH, S, D]
    return out.reshape(B, H, S, D).astype(np.float32)


if __name__ == "__main__":
    nc = build_program()
    print("program built ok")


# revision 37
# speedup vs baseline: 13059.6290x; 13059.6290x over previous
"""BigBird block-sparse attention on 8 Trainium2 NeuronCores.

Problem: B=2, H=16, S=4096, D=64, block 64, 3 random blocks/row, masks all-ones.
Sharding: the 32 (b,h) pairs are data-parallel; each of the 8 cores handles 4.

Per (b,h), query block-rows are processed in PAIRS (128 queries):
  - full pair   = rows (0, 63): full attention over all 4096 keys
  - special pair= rows (1, 62): blocks {0,1,2,63}+rand / {0,61,62,63}+rand
  - middle pairs l=0..29 = rows (2l+2, 2l+3): window + globals + 3 rand each

Scores are computed TRANSPOSED: S^T[k, q] = K @ Q^T with kT chunks as the
stationary operand, so exp() on the scalar engine writes P^T straight to
SBUF.  PV accumulates ctx^T[d, q] with V chunks (plus an appended ones
column that produces the softmax denominator) as weights.  One small PE
transpose per pair restores natural [q, d] layout; a reciprocal-scaled copy
normalizes.

Middle pairs pack their keys as [shared(2 chunks, full q) | per-half
chunks(2, half q)]: the A-half (first row) and B-half (second row) key
chunks share PSUM columns, so scores fit one PSUM bank, exp covers only
valid entries, and no masking is needed.

Random (and window-edge) K^T/V^T columns are gathered on the otherwise-idle
GPSIMD engine with one batched ap_gather per (b,h) over a combined
[K^T; V^T] table (SPMD-safe: indices are per-core input data).  The V^T
gathers are turned into natural-layout V chunks by PE transposes.

Softmax skips the max-subtraction: inputs are N(0,1) so scores/8 ~ N(0,1);
exp stays well inside fp32 range and matches the reference analytically.
"""

import dataclasses

import numpy as np

import concourse.bass as bass
import concourse.bacc as bacc
import concourse.mybir as mybir
import concourse.tile as tile

B, H, S, D = 2, 16, 4096, 64
BS = 64
NBLK = S // BS            # 64
NCORES = 8
NBH = (B * H) // NCORES   # 4 (b,h) pairs per core
NMID = 30                 # middle pairs per (b,h)
NG = 256                  # gathered slots per (b,h): 6 special + 8*30, padded
F16 = mybir.dt.float16
F32 = mybir.dt.float32
I16 = mybir.dt.int16
NEG = -30000.0


def _rows2(ap2, r1, r2, w):
    """[R, C] (DRAM) AP -> [2, w, C] view of row ranges {r1:r1+w, r2:r2+w}."""
    b = ap2[r1:r1 + w, :]
    row_step = b.ap[0][0]
    return dataclasses.replace(
        b, ap=[[(r2 - r1) * row_step, 2], list(b.ap[0]), list(b.ap[1])]
    )


def build_program():
    nc = bacc.Bacc(
        "TRN2",
        target_bir_lowering=False,
        debug=False,
        num_devices=NCORES,
    )
    qT_d = nc.dram_tensor("qT", [NBH, D, S], F16, kind="ExternalInput")
    # combined transposed table: partitions 0:64 = K^T, 64:128 = V^T
    kvT_d = nc.dram_tensor("kvT", [NBH, 128, S], F16, kind="ExternalInput")
    v_d = nc.dram_tensor("v", [NBH, S, D], F16, kind="ExternalInput")
    idxg_d = nc.dram_tensor("idxg", [NBH, 128, NG // 16], I16, kind="ExternalInput")
    id65_d = nc.dram_tensor("id65", [65, 65], F32, kind="ExternalInput")
    idhi_d = nc.dram_tensor("idhi", [128, 64], F16, kind="ExternalInput")
    out_d = nc.dram_tensor("out", [NBH, S, D], F32, kind="ExternalOutput")

    with tile.TileContext(nc) as tc:
        with (
            tc.tile_pool(name="const", bufs=1) as cpool,
            tc.tile_pool(name="bh", bufs=2) as bhpool,
            tc.tile_pool(name="work", bufs=3) as wpool,
            tc.tile_pool(name="fin", bufs=3) as fpool,
            tc.tile_pool(name="ps_s", bufs=2, space="PSUM") as ps_s,
            tc.tile_pool(name="ps_sp", bufs=1, space="PSUM") as ps_sp,
            tc.tile_pool(name="ps_ctxT", bufs=2, space="PSUM") as ps_ctxT,
            tc.tile_pool(name="ps_fin", bufs=2, space="PSUM") as ps_fin,
        ):
            id65 = cpool.tile([65, 65], F32)
            nc.sync.dma_start(out=id65[:], in_=id65_d[:])
            idhi = cpool.tile([128, 64], F16)  # identity on partitions 64:128
            nc.sync.dma_start(out=idhi[:], in_=idhi_d[:])

            for bh in range(NBH):
                qT_sb = bhpool.tile([D, S], F16, tag="qT")
                nc.sync.dma_start(out=qT_sb[:], in_=qT_d[bh])
                kvT_sb = bhpool.tile([128, S], F16, tag="kvT")
                nc.sync.dma_start(out=kvT_sb[:], in_=kvT_d[bh])

                # V stagings (natural layout), each with a ones-column
                vnat = bhpool.tile([128, 32, 65], F16, tag="vnat")
                nc.sync.dma_start(
                    out=vnat[:, :, 0:64],
                    in_=v_d[bh][:].rearrange("(m p) d -> p m d", p=128),
                )
                nc.vector.memset(vnat[:, :, 64:65], 1.0)
                vshift = bhpool.tile([128, 31, 65], F16, tag="vshift")
                nc.sync.dma_start(
                    out=vshift[:, :, 0:64],
                    in_=v_d[bh][64:4032, :].rearrange("(m p) d -> p m d", p=128),
                )
                nc.vector.memset(vshift[:, :, 64:65], 1.0)
                vg = bhpool.tile([128, 65], F16, tag="vg")
                nc.sync.dma_start(out=vg[:, 0:64], in_=_rows2(v_d[bh], 0, 4032, 64))
                nc.vector.memset(vg[:, 64:65], 1.0)
                vm0 = bhpool.tile([128, 65], F16, tag="vm0")
                nc.sync.dma_start(out=vm0[:, 0:64], in_=v_d[bh][0:128, :])
                nc.vector.memset(vm0[:, 64:65], 1.0)
                vm1 = bhpool.tile([128, 65], F16, tag="vm1")
                nc.sync.dma_start(out=vm1[:, 0:64], in_=_rows2(v_d[bh], 128, 4032, 64))
                nc.vector.memset(vm1[:, 64:65], 1.0)

                # batched K^T+V^T random/window-edge column gather on GPSIMD
                idxg_sb = bhpool.tile([128, NG // 16], I16, tag="idxg")
                nc.sync.dma_start(out=idxg_sb[:], in_=idxg_d[bh])
                ktr = bhpool.tile([128, NG * BS], F16, tag="ktr")
                nc.gpsimd.ap_gather(
                    out_ap=ktr[:],
                    in_ap=kvT_sb[:],
                    idxs_ap=idxg_sb[:],
                    channels=128,
                    num_elems=NBLK,
                    d=BS,
                    num_idxs=NG,
                )

                # contiguous stagings for the global (0, 63) kT columns
                ktg = bhpool.tile([D, 128], F16, tag="ktg")
                nc.vector.tensor_copy(ktg[:, 0:64], kvT_sb[0:64, 0:64])
                nc.vector.tensor_copy(ktg[:, 64:128], kvT_sb[0:64, 4032:4096])
                ktg2 = bhpool.tile([D, 128], F16, tag="ktg2")  # blocks (2, 63)
                nc.vector.tensor_copy(ktg2[:, 0:64], kvT_sb[0:64, 128:192])
                nc.vector.tensor_copy(ktg2[:, 64:128], kvT_sb[0:64, 4032:4096])

                # batched middle-pair output staging (one DMA per (b,h))
                ctx_all = bhpool.tile([128, NMID, 64], F32, tag="ctx_all")

                def finish_pair(ctxT, idhalf, out_spec):
                    """ctxT psum [65,128] -> transpose -> normalize -> out."""
                    ctxT_sb = fpool.tile([65, 128], F32, tag="ctxT_sb")
                    nc.vector.tensor_copy(ctxT_sb[:], ctxT[:])
                    ctx2 = ps_fin.tile([128, 65], F32, tag="fin")
                    nc.tensor.transpose(ctx2[:], ctxT_sb[:], id65[:])
                    recip = fpool.tile([128, 1], F32, tag="recip")
                    nc.vector.reciprocal(recip[:], ctx2[:, 64:65])
                    if out_spec[0] == "mid":
                        nc.scalar.activation(
                            out=ctx_all[:, out_spec[1], :], in_=ctx2[:, 0:64],
                            func=mybir.ActivationFunctionType.Copy,
                            scale=recip[:, 0:1],
                        )
                    else:
                        outsb = fpool.tile([128, 64], F32, tag="outsb")
                        nc.scalar.activation(
                            out=outsb[:], in_=ctx2[:, 0:64],
                            func=mybir.ActivationFunctionType.Copy,
                            scale=recip[:, 0:1],
                        )
                        nc.sync.dma_start(
                            out=_rows2(out_d[bh], out_spec[1], out_spec[2], 64),
                            in_=outsb[:],
                        )

                def make_vr(nslots, slot0, tag):
                    """PE-transpose gathered V^T slot pairs into natural-layout
                    V chunks [128, n, 65] with a ones column."""
                    n = nslots // 2
                    vtrT = ps_fin.tile([128, 4 * BS], F16, tag="fin")
                    for j in range(n):
                        base = BS * (slot0 + 2 * j)
                        nc.tensor.transpose(
                            vtrT[:, BS * j:BS * (j + 1)],
                            ktr[64:128, base:base + 128],
                            idhi[64:128, :],
                        )
                    vr = wpool.tile([128, 4, 65], F16, tag=tag)
                    nc.vector.tensor_copy(
                        vr[:, 0:n, 0:64],
                        vtrT[:, 0:n * BS].rearrange("p (a w) -> p a w", w=BS),
                    )
                    nc.vector.memset(vr[:, 0:n, 64:65], 1.0)
                    return vr

                # ---------------- full pair: rows 0 & 63 ----------------
                qts = wpool.tile([D, 128], F16, tag="qts")
                nc.vector.tensor_copy(qts[:, 0:64], qT_sb[:, 0:64])
                nc.vector.tensor_copy(qts[:, 64:128], qT_sb[:, 4032:4096])
                chunks = [(ktg[:], vg[:])]
                chunks += [
                    (kvT_sb[0:64, 64 + 128 * m:192 + 128 * m], vshift[:, m, :])
                    for m in range(31)
                ]
                ctxT = ps_ctxT.tile([65, 128], F32, tag="ctxT")
                ci = 0
                for g0 in range(0, 32, 4):
                    gch = chunks[g0:g0 + 4]
                    s_ps = ps_s.tile([128, 4, 128], F32, tag="s")
                    pt = wpool.tile([128, 4, 128], F16, tag="pt")
                    for j, (kap, _) in enumerate(gch):
                        nc.tensor.matmul(out=s_ps[:, j, :], lhsT=kap, rhs=qts[:],
                                         start=True, stop=True)
                    nc.scalar.activation(
                        out=pt[:], in_=s_ps[:],
                        func=mybir.ActivationFunctionType.Exp, scale=0.125)
                    for j, (_, vap) in enumerate(gch):
                        nc.tensor.matmul(out=ctxT[:], lhsT=vap, rhs=pt[:, j, :],
                                         start=(ci == 0), stop=(ci == 31),
                                         skip_group_check=True)
                        ci += 1
                finish_pair(ctxT, None, ("two", 0, 4032))

                # ---------------- special pair: rows 1 & 62 ----------------
                qts = wpool.tile([D, 128], F16, tag="qts")
                nc.vector.tensor_copy(qts[:, 0:64], qT_sb[:, 64:128])
                nc.vector.tensor_copy(qts[:, 64:128], qT_sb[:, 3968:4032])
                vr = make_vr(6, 0, "vr_sp")
                chunks = [
                    (kvT_sb[0:64, 0:128], vm0[:]),       # blocks (0, 1)
                    (ktg2[:], vm1[:]),                   # blocks (2, 63)
                    (kvT_sb[0:64, 3904:4032], vshift[:, 30, :]),  # (61, 62)
                    (ktr[0:64, 0:128], vr[:, 0, :]),     # (rA0, rA1)
                    (ktr[0:64, 128:256], vr[:, 1, :]),   # (rA2, rB0)
                    (ktr[0:64, 256:384], vr[:, 2, :]),   # (rB1, rB2)
                ]
                s_ps = ps_sp.tile([128, 6, 128], F32, tag="ssp")
                pt = wpool.tile([128, 6, 128], F16, tag="pt_sp")
                for j, (kap, _) in enumerate(chunks):
                    nc.tensor.matmul(out=s_ps[:, j, :], lhsT=kap, rhs=qts[:],
                                     start=True, stop=True)
                nc.scalar.activation(
                    out=pt[:], in_=s_ps[:],
                    func=mybir.ActivationFunctionType.Exp, scale=0.125)
                # zero invalid P^T entries (row62 must not see A's blocks etc)
                for (psl, cj, csl) in [
                    (slice(64, 128), 0, slice(64, 128)),  # blk1 not for row62
                    (slice(0, 64), 1, slice(64, 128)),    # blk2 not for row62
                    (slice(0, 128), 2, slice(0, 64)),     # 61,62 not for row1
                    (slice(0, 128), 3, slice(64, 128)),   # randA not for row62
                    (slice(0, 64), 4, slice(64, 128)),    # rA2 not for row62
                    (slice(64, 128), 4, slice(0, 64)),    # rB0 not for row1
                    (slice(0, 128), 5, slice(0, 64)),     # randB not for row1
                ]:
                    nc.vector.memset(pt[psl, cj, csl], 0.0)
                ctxT = ps_ctxT.tile([65, 128], F32, tag="ctxT")
                for j, (_, vap) in enumerate(chunks):
                    nc.tensor.matmul(out=ctxT[:], lhsT=vap, rhs=pt[:, j, :],
                                     start=(j == 0), stop=(j == 5),
                                     skip_group_check=True)
                finish_pair(ctxT, None, ("two", 64, 3968))

                # ---------------- middle pairs ----------------
                for l in range(NMID):
                    sl0 = 6 + 8 * l
                    q_ap = qT_sb[:, 128 * (l + 1):128 * (l + 2)]
                    qA = qT_sb[:, 128 * (l + 1):128 * (l + 1) + 64]
                    qB = qT_sb[:, 128 * (l + 1) + 64:128 * (l + 2)]
                    vr = make_vr(8, sl0, "vr")
                    s_ps = ps_s.tile([128, 4, 128], F32, tag="s")
                    pt = wpool.tile([128, 4, 128], F16, tag="pt")
                    # shared chunks: window middle (2l+2, 2l+3) and globals
                    nc.tensor.matmul(
                        out=s_ps[:, 0, :],
                        lhsT=kvT_sb[0:64, 128 * (l + 1):128 * (l + 2)],
                        rhs=q_ap, start=True, stop=True)
                    nc.tensor.matmul(
                        out=s_ps[:, 1, :], lhsT=ktg[:], rhs=q_ap,
                        start=True, stop=True)
                    # per-half chunks share PSUM columns: A in q-cols 0:64,
                    # B in 64:128
                    kb = BS * sl0
                    nc.tensor.matmul(out=s_ps[:, 2, 0:64],
                                     lhsT=ktr[0:64, kb:kb + 128],
                                     rhs=qA, start=True, stop=True)
                    nc.tensor.matmul(out=s_ps[:, 3, 0:64],
                                     lhsT=ktr[0:64, kb + 128:kb + 256],
                                     rhs=qA, start=True, stop=True)
                    nc.tensor.matmul(out=s_ps[:, 2, 64:128],
                                     lhsT=ktr[0:64, kb + 256:kb + 384],
                                     rhs=qB, start=True, stop=True)
                    nc.tensor.matmul(out=s_ps[:, 3, 64:128],
                                     lhsT=ktr[0:64, kb + 384:kb + 512],
                                     rhs=qB, start=True, stop=True)
                    nc.scalar.activation(
                        out=pt[:], in_=s_ps[:],
                        func=mybir.ActivationFunctionType.Exp, scale=0.125)
                    ctxT = ps_ctxT.tile([65, 128], F32, tag="ctxT")
                    nc.tensor.matmul(out=ctxT[:], lhsT=vnat[:, l + 1, :],
                                     rhs=pt[:, 0, :], start=True, stop=False,
                                     skip_group_check=True)
                    nc.tensor.matmul(out=ctxT[:], lhsT=vg[:],
                                     rhs=pt[:, 1, :], start=False, stop=False,
                                     skip_group_check=True)
                    nc.tensor.matmul(out=ctxT[:, 0:64], lhsT=vr[:, 0, :],
                                     rhs=pt[:, 2, 0:64], start=False, stop=False,
                                     skip_group_check=True)
                    nc.tensor.matmul(out=ctxT[:, 0:64], lhsT=vr[:, 1, :],
                                     rhs=pt[:, 3, 0:64], start=False, stop=False,
                                     skip_group_check=True)
                    nc.tensor.matmul(out=ctxT[:, 64:128], lhsT=vr[:, 2, :],
                                     rhs=pt[:, 2, 64:128], start=False, stop=False,
                                     skip_group_check=True)
                    nc.tensor.matmul(out=ctxT[:, 64:128], lhsT=vr[:, 3, :],
                                     rhs=pt[:, 3, 64:128], start=False, stop=True,
                                     skip_group_check=True)
                    finish_pair(ctxT, None, ("mid", l))

                # one batched output DMA for the 30 middle pairs
                nc.sync.dma_start(
                    out=out_d[bh][128:3968, :].rearrange(
                        "(m p) d -> p m d", p=128),
                    in_=ctx_all[:],
                )

    nc.compile()
    return nc


_PROGRAM = None


def _get_program():
    global _PROGRAM
    if _PROGRAM is None:
        _PROGRAM = build_program()
    return _PROGRAM


def make_core_inputs(q, k, v, rand, bh_slice):
    """Build one core's input map from full [32, S, D] arrays (fp32)."""
    qs = q[bh_slice]
    ks = k[bh_slice]
    vs = v[bh_slice]
    rs = rand[bh_slice]  # [NBH, 62, 3]
    qT = np.ascontiguousarray(qs.transpose(0, 2, 1)).astype(np.float16)
    kvT = np.concatenate(
        [qs.transpose(0, 2, 1) * 0, qs.transpose(0, 2, 1) * 0], axis=1
    ).astype(np.float16)
    kvT[:, 0:64, :] = ks.transpose(0, 2, 1).astype(np.float16)
    kvT[:, 64:128, :] = vs.transpose(0, 2, 1).astype(np.float16)
    vv = vs.astype(np.float16)

    # gather slot list per (b,h): 6 special + 8 per middle pair, pad to NG
    slots = np.zeros((NBH, NG), np.int16)
    for n in range(NBH):
        ra, rb = rs[n, 0], rs[n, 61]
        slots[n, 0:6] = np.concatenate([ra, rb])      # special: rA(3), rB(3)
        for l in range(NMID):
            sl0 = 6 + 8 * l
            ra, rb = rs[n, 2 * l + 1], rs[n, 2 * l + 2]
            slots[n, sl0] = 2 * l + 1                 # window edge A
            slots[n, sl0 + 1:sl0 + 4] = ra
            slots[n, sl0 + 4] = 2 * l + 4             # window edge B
            slots[n, sl0 + 5:sl0 + 8] = rb
    # ap_gather wrapped layout: index i -> partition 16g + i%16, col i//16
    idxg = np.empty((NBH, 128, NG // 16), np.int16)
    for n in range(NBH):
        wrapped = slots[n].reshape(NG // 16, 16).T    # [16, NG//16]
        idxg[n] = np.tile(wrapped, (8, 1))

    id65 = np.eye(65, dtype=np.float32)
    idhi = np.zeros((128, 64), np.float16)
    idhi[64:128, :] = np.eye(64, dtype=np.float16)
    return {"qT": qT, "kvT": kvT, "v": vv, "idxg": idxg,
            "id65": id65, "idhi": idhi}


def kernel(query, key, value, from_blocked_mask=None, to_blocked_mask=None,
           rand_attn=None, **_ignored):
    # masks are all-ones in this problem's input distribution; the block
    # structure (window/global/random) is handled explicitly.
    q = np.asarray(query, np.float32).reshape(B * H, S, D)
    k = np.asarray(key, np.float32).reshape(B * H, S, D)
    v = np.asarray(value, np.float32).reshape(B * H, S, D)
    rand = np.asarray(rand_attn).reshape(B * H, NBLK - 2, 3).astype(np.int32)

    in_maps = [
        make_core_inputs(q, k, v, rand, slice(c * NBH, (c + 1) * NBH))
        for c in range(NCORES)
    ]
    nc = _get_program()
    from concourse import bass_utils
    res = bass_utils.run_bass_kernel_spmd(nc, in_maps, core_ids=list(range(NCORES)))
    out = np.stack([r["out"] for r in res.results])  # [8, NBH, S, D]
    return out.reshape(B, H, S, D).astype(np.float32)


if __name__ == "__main__":
    nc = build_program()
    print("program built ok")


# revision 42
# speedup vs baseline: 13465.9691x; 1.0311x over previous
"""BigBird block-sparse attention on 8 Trainium2 NeuronCores.

Problem: B=2, H=16, S=4096, D=64, block 64, 3 random blocks/row, masks all-ones.
Sharding: the 32 (b,h) pairs are data-parallel; each of the 8 cores handles 4.

Per (b,h), query block-rows are processed in PAIRS (128 queries):
  - full pair   = rows (0, 63): full attention over all 4096 keys
  - special pair= rows (1, 62): blocks {0,1,2,63}+rand / {0,61,62,63}+rand
  - middle pairs l=0..29 = rows (2l+2, 2l+3): window + globals + 3 rand each

Scores are computed TRANSPOSED: S^T[k, q] = K @ Q^T with kT chunks as the
stationary operand, so exp() on the scalar engine writes P^T straight to
SBUF.  PV accumulates ctx^T[d, q] with V chunks (plus an appended ones
column that produces the softmax denominator) as weights.  One small PE
transpose per pair restores natural [q, d] layout; a reciprocal-scaled copy
normalizes.

Middle pairs pack their keys as [shared(2 chunks, full q) | per-half
chunks(2, half q)]: the A-half (first row) and B-half (second row) key
chunks share PSUM columns, so scores fit one PSUM bank, exp covers only
valid entries, and no masking is needed.

Random (and window-edge) K^T/V^T columns are gathered on the otherwise-idle
GPSIMD engine with one batched ap_gather per (b,h) over a combined
[K^T; V^T] table (SPMD-safe: indices are per-core input data).  The V^T
gathers are turned into natural-layout V chunks by PE transposes.

Softmax skips the max-subtraction: inputs are N(0,1) so scores/8 ~ N(0,1);
exp stays well inside fp32 range and matches the reference analytically.
"""

import dataclasses

import numpy as np

import concourse.bass as bass
import concourse.bacc as bacc
import concourse.mybir as mybir
import concourse.tile as tile

B, H, S, D = 2, 16, 4096, 64
BS = 64
NBLK = S // BS            # 64
NCORES = 8
NBH = (B * H) // NCORES   # 4 (b,h) pairs per core
NMID = 30                 # middle pairs per (b,h)
NG = 256                  # gathered slots per (b,h): 6 special + 8*30, padded
F16 = mybir.dt.float16
F32 = mybir.dt.float32
I16 = mybir.dt.int16
NEG = -30000.0


def _rows2(ap2, r1, r2, w):
    """[R, C] (DRAM) AP -> [2, w, C] view of row ranges {r1:r1+w, r2:r2+w}."""
    b = ap2[r1:r1 + w, :]
    row_step = b.ap[0][0]
    return dataclasses.replace(
        b, ap=[[(r2 - r1) * row_step, 2], list(b.ap[0]), list(b.ap[1])]
    )


def build_program():
    nc = bacc.Bacc(
        "TRN2",
        target_bir_lowering=False,
        debug=False,
        num_devices=NCORES,
    )
    qT_d = nc.dram_tensor("qT", [NBH, D, S], F16, kind="ExternalInput")
    # combined transposed table: partitions 0:64 = K^T, 64:128 = V^T
    kvT_d = nc.dram_tensor("kvT", [NBH, 128, S], F16, kind="ExternalInput")
    v_d = nc.dram_tensor("v", [NBH, S, D], F16, kind="ExternalInput")
    idxg_d = nc.dram_tensor("idxg", [NBH, 128, NG // 16], I16, kind="ExternalInput")
    id65_d = nc.dram_tensor("id65", [65, 65], F32, kind="ExternalInput")
    idhi_d = nc.dram_tensor("idhi", [128, 64], F16, kind="ExternalInput")
    out_d = nc.dram_tensor("out", [NBH, S, D], F32, kind="ExternalOutput")

    with tile.TileContext(nc) as tc:
        with (
            tc.tile_pool(name="const", bufs=1) as cpool,
            tc.tile_pool(name="bh", bufs=2) as bhpool,
            tc.tile_pool(name="work", bufs=4) as wpool,
            tc.tile_pool(name="fin", bufs=4) as fpool,
            tc.tile_pool(name="ps_s", bufs=3, space="PSUM") as ps_s,
            tc.tile_pool(name="ps_ctxT", bufs=3, space="PSUM") as ps_ctxT,
            tc.tile_pool(name="ps_fin", bufs=2, space="PSUM") as ps_fin,
        ):
            id65 = cpool.tile([65, 65], F32)
            nc.sync.dma_start(out=id65[:], in_=id65_d[:])
            idhi = cpool.tile([128, 64], F16)  # identity on partitions 64:128
            nc.sync.dma_start(out=idhi[:], in_=idhi_d[:])

            for bh in range(NBH):
                qT_sb = bhpool.tile([D, S], F16, tag="qT")
                nc.sync.dma_start(out=qT_sb[:], in_=qT_d[bh])
                kvT_sb = bhpool.tile([128, S], F16, tag="kvT")
                nc.sync.dma_start(out=kvT_sb[:], in_=kvT_d[bh])

                # V stagings (natural layout), each with a ones-column
                vnat = bhpool.tile([128, 32, 65], F16, tag="vnat")
                nc.sync.dma_start(
                    out=vnat[:, :, 0:64],
                    in_=v_d[bh][:].rearrange("(m p) d -> p m d", p=128),
                )
                nc.vector.memset(vnat[:, :, 64:65], 1.0)
                vshift = bhpool.tile([128, 31, 65], F16, tag="vshift")
                nc.sync.dma_start(
                    out=vshift[:, :, 0:64],
                    in_=v_d[bh][64:4032, :].rearrange("(m p) d -> p m d", p=128),
                )
                nc.vector.memset(vshift[:, :, 64:65], 1.0)
                vg = bhpool.tile([128, 65], F16, tag="vg")
                nc.sync.dma_start(out=vg[:, 0:64], in_=_rows2(v_d[bh], 0, 4032, 64))
                nc.vector.memset(vg[:, 64:65], 1.0)
                vm0 = bhpool.tile([128, 65], F16, tag="vm0")
                nc.sync.dma_start(out=vm0[:, 0:64], in_=v_d[bh][0:128, :])
                nc.vector.memset(vm0[:, 64:65], 1.0)
                vm1 = bhpool.tile([128, 65], F16, tag="vm1")
                nc.sync.dma_start(out=vm1[:, 0:64], in_=_rows2(v_d[bh], 128, 4032, 64))
                nc.vector.memset(vm1[:, 64:65], 1.0)

                # batched K^T+V^T random/window-edge column gather on GPSIMD
                idxg_sb = bhpool.tile([128, NG // 16], I16, tag="idxg")
                nc.sync.dma_start(out=idxg_sb[:], in_=idxg_d[bh])
                ktr = bhpool.tile([128, NG * BS], F16, tag="ktr")
                nc.gpsimd.ap_gather(
                    out_ap=ktr[:],
                    in_ap=kvT_sb[:],
                    idxs_ap=idxg_sb[:],
                    channels=128,
                    num_elems=NBLK,
                    d=BS,
                    num_idxs=NG,
                )

                # contiguous stagings for the global (0, 63) kT columns
                ktg = bhpool.tile([D, 128], F16, tag="ktg")
                nc.vector.tensor_copy(ktg[:, 0:64], kvT_sb[0:64, 0:64])
                nc.vector.tensor_copy(ktg[:, 64:128], kvT_sb[0:64, 4032:4096])
                ktg2 = bhpool.tile([D, 128], F16, tag="ktg2")  # blocks (2, 63)
                nc.vector.tensor_copy(ktg2[:, 0:64], kvT_sb[0:64, 128:192])
                nc.vector.tensor_copy(ktg2[:, 64:128], kvT_sb[0:64, 4032:4096])

                # batched middle-pair output staging (one DMA per (b,h))
                ctx_all = bhpool.tile([128, NMID, 64], F32, tag="ctx_all")

                def finish_pair(ctxT, idhalf, out_spec):
                    """ctxT psum [65,128] -> transpose -> normalize -> out."""
                    ctxT_sb = fpool.tile([65, 128], F32, tag="ctxT_sb")
                    nc.vector.tensor_copy(ctxT_sb[:], ctxT[:])
                    ctx2 = ps_fin.tile([128, 65], F32, tag="fin")
                    nc.tensor.transpose(ctx2[:], ctxT_sb[:], id65[:])
                    recip = fpool.tile([128, 1], F32, tag="recip")
                    nc.vector.reciprocal(recip[:], ctx2[:, 64:65])
                    if out_spec[0] == "mid":
                        # alternate the normalize between ACT and DVE to
                        # balance engine load (ACT also runs every exp)
                        if out_spec[1] % 2 == 0:
                            nc.scalar.activation(
                                out=ctx_all[:, out_spec[1], :], in_=ctx2[:, 0:64],
                                func=mybir.ActivationFunctionType.Copy,
                                scale=recip[:, 0:1],
                            )
                        else:
                            nc.vector.tensor_scalar_mul(
                                ctx_all[:, out_spec[1], :], ctx2[:, 0:64],
                                recip[:, 0:1],
                            )
                    else:
                        outsb = fpool.tile([128, 64], F32, tag="outsb")
                        nc.scalar.activation(
                            out=outsb[:], in_=ctx2[:, 0:64],
                            func=mybir.ActivationFunctionType.Copy,
                            scale=recip[:, 0:1],
                        )
                        nc.sync.dma_start(
                            out=_rows2(out_d[bh], out_spec[1], out_spec[2], 64),
                            in_=outsb[:],
                        )

                def make_vr(nslots, slot0, tag):
                    """PE-transpose gathered V^T slot pairs into natural-layout
                    V chunks [128, n, 65] with a ones column."""
                    n = nslots // 2
                    vtrT = ps_fin.tile([128, 4 * BS], F16, tag="fin")
                    for j in range(n):
                        base = BS * (slot0 + 2 * j)
                        nc.tensor.transpose(
                            vtrT[:, BS * j:BS * (j + 1)],
                            ktr[64:128, base:base + 128],
                            idhi[64:128, :],
                        )
                    vr = wpool.tile([128, 4, 65], F16, tag=tag)
                    nc.vector.tensor_copy(
                        vr[:, 0:n, 0:64],
                        vtrT[:, 0:n * BS].rearrange("p (a w) -> p a w", w=BS),
                    )
                    nc.vector.memset(vr[:, 0:n, 64:65], 1.0)
                    return vr

                # ---------------- full pair: rows 0 & 63 ----------------
                qts = wpool.tile([D, 128], F16, tag="qts")
                nc.vector.tensor_copy(qts[:, 0:64], qT_sb[:, 0:64])
                nc.vector.tensor_copy(qts[:, 64:128], qT_sb[:, 4032:4096])
                chunks = [(ktg[:], vg[:])]
                chunks += [
                    (kvT_sb[0:64, 64 + 128 * m:192 + 128 * m], vshift[:, m, :])
                    for m in range(31)
                ]
                ctxT = ps_ctxT.tile([65, 128], F32, tag="ctxT")
                ci = 0
                for g0 in range(0, 32, 4):
                    gch = chunks[g0:g0 + 4]
                    s_ps = ps_s.tile([128, 4, 128], F32, tag="s")
                    pt = wpool.tile([128, 4, 128], F16, tag="pt")
                    for j, (kap, _) in enumerate(gch):
                        nc.tensor.matmul(out=s_ps[:, j, :], lhsT=kap, rhs=qts[:],
                                         start=True, stop=True)
                    nc.scalar.activation(
                        out=pt[:], in_=s_ps[:],
                        func=mybir.ActivationFunctionType.Exp, scale=0.125)
                    for j, (_, vap) in enumerate(gch):
                        nc.tensor.matmul(out=ctxT[:], lhsT=vap, rhs=pt[:, j, :],
                                         start=(ci == 0), stop=(ci == 31),
                                         skip_group_check=True)
                        ci += 1
                finish_pair(ctxT, None, ("two", 0, 4032))

                # ---------------- special pair: rows 1 & 62 ----------------
                qts = wpool.tile([D, 128], F16, tag="qts")
                nc.vector.tensor_copy(qts[:, 0:64], qT_sb[:, 64:128])
                nc.vector.tensor_copy(qts[:, 64:128], qT_sb[:, 3968:4032])
                vr = make_vr(6, 0, "vr_sp")
                chunks = [
                    (kvT_sb[0:64, 0:128], vm0[:]),       # blocks (0, 1)
                    (ktg2[:], vm1[:]),                   # blocks (2, 63)
                    (kvT_sb[0:64, 3904:4032], vshift[:, 30, :]),  # (61, 62)
                    (ktr[0:64, 0:128], vr[:, 0, :]),     # (rA0, rA1)
                    (ktr[0:64, 128:256], vr[:, 1, :]),   # (rA2, rB0)
                    (ktr[0:64, 256:384], vr[:, 2, :]),   # (rB1, rB2)
                ]
                # zero invalid P^T entries (row62 must not see A's blocks etc)
                sp_masks = [
                    (slice(64, 128), 0, slice(64, 128)),  # blk1 not for row62
                    (slice(0, 64), 1, slice(64, 128)),    # blk2 not for row62
                    (slice(0, 128), 2, slice(0, 64)),     # 61,62 not for row1
                    (slice(0, 128), 3, slice(64, 128)),   # randA not for row62
                    (slice(0, 64), 0, slice(64, 128), 1),  # rA2 not for row62
                    (slice(64, 128), 0, slice(0, 64), 1),  # rB0 not for row1
                    (slice(0, 128), 1, slice(0, 64), 1),   # randB not for row1
                ]
                ctxT = ps_ctxT.tile([65, 128], F32, tag="ctxT")
                ci = 0
                for g0 in range(0, 6, 4):
                    gch = chunks[g0:g0 + 4]
                    gi = g0 // 4
                    s_ps = ps_s.tile([128, 4, 128], F32, tag="s")
                    pt = wpool.tile([128, 4, 128], F16, tag="pt")
                    for j, (kap, _) in enumerate(gch):
                        nc.tensor.matmul(out=s_ps[:, j, :], lhsT=kap, rhs=qts[:],
                                         start=True, stop=True)
                    nc.scalar.activation(
                        out=pt[:, 0:len(gch), :], in_=s_ps[:, 0:len(gch), :],
                        func=mybir.ActivationFunctionType.Exp, scale=0.125)
                    for m in sp_masks:
                        if (m[3] if len(m) > 3 else 0) == gi:
                            nc.vector.memset(pt[m[0], m[1], m[2]], 0.0)
                    for j, (_, vap) in enumerate(gch):
                        nc.tensor.matmul(out=ctxT[:], lhsT=vap, rhs=pt[:, j, :],
                                         start=(ci == 0), stop=(ci == 5),
                                         skip_group_check=True)
                        ci += 1
                finish_pair(ctxT, None, ("two", 64, 3968))

                # ---------------- middle pairs ----------------
                for l in range(NMID):
                    sl0 = 6 + 8 * l
                    q_ap = qT_sb[:, 128 * (l + 1):128 * (l + 2)]
                    qA = qT_sb[:, 128 * (l + 1):128 * (l + 1) + 64]
                    qB = qT_sb[:, 128 * (l + 1) + 64:128 * (l + 2)]
                    vr = make_vr(8, sl0, "vr")
                    s_ps = ps_s.tile([128, 4, 128], F32, tag="s")
                    pt = wpool.tile([128, 4, 128], F16, tag="pt")
                    # shared chunks: window middle (2l+2, 2l+3) and globals
                    nc.tensor.matmul(
                        out=s_ps[:, 0, :],
                        lhsT=kvT_sb[0:64, 128 * (l + 1):128 * (l + 2)],
                        rhs=q_ap, start=True, stop=True)
                    nc.tensor.matmul(
                        out=s_ps[:, 1, :], lhsT=ktg[:], rhs=q_ap,
                        start=True, stop=True)
                    # per-half chunks share PSUM columns: A in q-cols 0:64,
                    # B in 64:128
                    kb = BS * sl0
                    nc.tensor.matmul(out=s_ps[:, 2, 0:64],
                                     lhsT=ktr[0:64, kb:kb + 128],
                                     rhs=qA, start=True, stop=True)
                    nc.tensor.matmul(out=s_ps[:, 3, 0:64],
                                     lhsT=ktr[0:64, kb + 128:kb + 256],
                                     rhs=qA, start=True, stop=True)
                    nc.tensor.matmul(out=s_ps[:, 2, 64:128],
                                     lhsT=ktr[0:64, kb + 256:kb + 384],
                                     rhs=qB, start=True, stop=True)
                    nc.tensor.matmul(out=s_ps[:, 3, 64:128],
                                     lhsT=ktr[0:64, kb + 384:kb + 512],
                                     rhs=qB, start=True, stop=True)
                    nc.scalar.activation(
                        out=pt[:], in_=s_ps[:],
                        func=mybir.ActivationFunctionType.Exp, scale=0.125)
                    ctxT = ps_ctxT.tile([65, 128], F32, tag="ctxT")
                    nc.tensor.matmul(out=ctxT[:], lhsT=vnat[:, l + 1, :],
                                     rhs=pt[:, 0, :], start=True, stop=False,
                                     skip_group_check=True)
                    nc.tensor.matmul(out=ctxT[:], lhsT=vg[:],
                                     rhs=pt[:, 1, :], start=False, stop=False,
                                     skip_group_check=True)
                    nc.tensor.matmul(out=ctxT[:, 0:64], lhsT=vr[:, 0, :],
                                     rhs=pt[:, 2, 0:64], start=False, stop=False,
                                     skip_group_check=True)
                    nc.tensor.matmul(out=ctxT[:, 0:64], lhsT=vr[:, 1, :],
                                     rhs=pt[:, 3, 0:64], start=False, stop=False,
                                     skip_group_check=True)
                    nc.tensor.matmul(out=ctxT[:, 64:128], lhsT=vr[:, 2, :],
                                     rhs=pt[:, 2, 64:128], start=False, stop=False,
                                     skip_group_check=True)
                    nc.tensor.matmul(out=ctxT[:, 64:128], lhsT=vr[:, 3, :],
                                     rhs=pt[:, 3, 64:128], start=False, stop=True,
                                     skip_group_check=True)
                    finish_pair(ctxT, None, ("mid", l))

                # one batched output DMA for the 30 middle pairs
                nc.sync.dma_start(
                    out=out_d[bh][128:3968, :].rearrange(
                        "(m p) d -> p m d", p=128),
                    in_=ctx_all[:],
                )

    nc.compile()
    return nc


_PROGRAM = None


def _get_program():
    global _PROGRAM
    if _PROGRAM is None:
        _PROGRAM = build_program()
    return _PROGRAM


def make_core_inputs(q, k, v, rand, bh_slice):
    """Build one core's input map from full [32, S, D] arrays (fp32)."""
    qs = q[bh_slice]
    ks = k[bh_slice]
    vs = v[bh_slice]
    rs = rand[bh_slice]  # [NBH, 62, 3]
    qT = np.ascontiguousarray(qs.transpose(0, 2, 1)).astype(np.float16)
    kvT = np.concatenate(
        [qs.transpose(0, 2, 1) * 0, qs.transpose(0, 2, 1) * 0], axis=1
    ).astype(np.float16)
    kvT[:, 0:64, :] = ks.transpose(0, 2, 1).astype(np.float16)
    kvT[:, 64:128, :] = vs.transpose(0, 2, 1).astype(np.float16)
    vv = vs.astype(np.float16)

    # gather slot list per (b,h): 6 special + 8 per middle pair, pad to NG
    slots = np.zeros((NBH, NG), np.int16)
    for n in range(NBH):
        ra, rb = rs[n, 0], rs[n, 61]
        slots[n, 0:6] = np.concatenate([ra, rb])      # special: rA(3), rB(3)
        for l in range(NMID):
            sl0 = 6 + 8 * l
            ra, rb = rs[n, 2 * l + 1], rs[n, 2 * l + 2]
            slots[n, sl0] = 2 * l + 1                 # window edge A
            slots[n, sl0 + 1:sl0 + 4] = ra
            slots[n, sl0 + 4] = 2 * l + 4             # window edge B
            slots[n, sl0 + 5:sl0 + 8] = rb
    # ap_gather wrapped layout: index i -> partition 16g + i%16, col i//16
    idxg = np.empty((NBH, 128, NG // 16), np.int16)
    for n in range(NBH):
        wrapped = slots[n].reshape(NG // 16, 16).T    # [16, NG//16]
        idxg[n] = np.tile(wrapped, (8, 1))

    id65 = np.eye(65, dtype=np.float32)
    idhi = np.zeros((128, 64), np.float16)
    idhi[64:128, :] = np.eye(64, dtype=np.float16)
    return {"qT": qT, "kvT": kvT, "v": vv, "idxg": idxg,
            "id65": id65, "idhi": idhi}


def kernel(query, key, value, from_blocked_mask=None, to_blocked_mask=None,
           rand_attn=None, **_ignored):
    # masks are all-ones in this problem's input distribution; the block
    # structure (window/global/random) is handled explicitly.
    q = np.asarray(query, np.float32).reshape(B * H, S, D)
    k = np.asarray(key, np.float32).reshape(B * H, S, D)
    v = np.asarray(value, np.float32).reshape(B * H, S, D)
    rand = np.asarray(rand_attn).reshape(B * H, NBLK - 2, 3).astype(np.int32)

    in_maps = [
        make_core_inputs(q, k, v, rand, slice(c * NBH, (c + 1) * NBH))
        for c in range(NCORES)
    ]
    nc = _get_program()
    from concourse import bass_utils
    res = bass_utils.run_bass_kernel_spmd(nc, in_maps, core_ids=list(range(NCORES)))
    out = np.stack([r["out"] for r in res.results])  # [8, NBH, S, D]
    return out.reshape(B, H, S, D).astype(np.float32)


if __name__ == "__main__":
    nc = build_program()
    print("program built ok")


# revision 43
# speedup vs baseline: 14106.1082x; 1.0475x over previous
"""BigBird block-sparse attention on 8 Trainium2 NeuronCores.

Problem: B=2, H=16, S=4096, D=64, block 64, 3 random blocks/row, masks all-ones.
Sharding: the 32 (b,h) pairs are data-parallel; each of the 8 cores handles 4.

Per (b,h), query block-rows are processed in PAIRS (128 queries):
  - full pair   = rows (0, 63): full attention over all 4096 keys
  - special pair= rows (1, 62): blocks {0,1,2,63}+rand / {0,61,62,63}+rand
  - middle pairs l=0..29 = rows (2l+2, 2l+3): window + globals + 3 rand each

Scores are computed TRANSPOSED: S^T[k, q] = K @ Q^T with kT chunks as the
stationary operand, so exp() on the scalar engine writes P^T straight to
SBUF.  PV accumulates ctx^T[d, q] with V chunks (plus an appended ones
column that produces the softmax denominator) as weights.  One small PE
transpose per pair restores natural [q, d] layout; a reciprocal-scaled copy
normalizes.

Middle pairs pack their keys as [shared(2 chunks, full q) | per-half
chunks(2, half q)]: the A-half (first row) and B-half (second row) key
chunks share PSUM columns, so scores fit one PSUM bank, exp covers only
valid entries, and no masking is needed.

Random (and window-edge) K^T/V^T columns are gathered on the otherwise-idle
GPSIMD engine with one batched ap_gather per (b,h) over a combined
[K^T; V^T] table (SPMD-safe: indices are per-core input data).  The V^T
gathers are turned into natural-layout V chunks by PE transposes.

Softmax skips the max-subtraction: inputs are N(0,1) so scores/8 ~ N(0,1);
exp stays well inside fp32 range and matches the reference analytically.
"""

import dataclasses

import numpy as np

import concourse.bass as bass
import concourse.bacc as bacc
import concourse.mybir as mybir
import concourse.tile as tile

B, H, S, D = 2, 16, 4096, 64
BS = 64
NBLK = S // BS            # 64
NCORES = 8
NBH = (B * H) // NCORES   # 4 (b,h) pairs per core
NMID = 30                 # middle pairs per (b,h)
NG = 256                  # gathered slots per (b,h): 6 special + 8*30, padded
F16 = mybir.dt.float16
F32 = mybir.dt.float32
I16 = mybir.dt.int16
NEG = -30000.0


def _rows2(ap2, r1, r2, w):
    """[R, C] (DRAM) AP -> [2, w, C] view of row ranges {r1:r1+w, r2:r2+w}."""
    b = ap2[r1:r1 + w, :]
    row_step = b.ap[0][0]
    return dataclasses.replace(
        b, ap=[[(r2 - r1) * row_step, 2], list(b.ap[0]), list(b.ap[1])]
    )


def build_program():
    nc = bacc.Bacc(
        "TRN2",
        target_bir_lowering=False,
        debug=False,
        num_devices=NCORES,
    )
    qT_d = nc.dram_tensor("qT", [NBH, D, S], F16, kind="ExternalInput")
    # combined transposed table: partitions 0:64 = K^T, 64:128 = V^T
    kvT_d = nc.dram_tensor("kvT", [NBH, 128, S], F16, kind="ExternalInput")
    v_d = nc.dram_tensor("v", [NBH, S, D], F16, kind="ExternalInput")
    idxg_d = nc.dram_tensor("idxg", [NBH, 128, NG // 16], I16, kind="ExternalInput")
    id65_d = nc.dram_tensor("id65", [65, 65], F32, kind="ExternalInput")
    idhi_d = nc.dram_tensor("idhi", [128, 64], F16, kind="ExternalInput")
    out_d = nc.dram_tensor("out", [NBH, S, D], F32, kind="ExternalOutput")

    with tile.TileContext(nc) as tc:
        with (
            tc.tile_pool(name="const", bufs=1) as cpool,
            tc.tile_pool(name="bh", bufs=2) as bhpool,
            tc.tile_pool(name="work", bufs=4) as wpool,
            tc.tile_pool(name="fin", bufs=4) as fpool,
            tc.tile_pool(name="ps_s", bufs=3, space="PSUM") as ps_s,
            tc.tile_pool(name="ps_ctxT", bufs=3, space="PSUM") as ps_ctxT,
            tc.tile_pool(name="ps_fin", bufs=2, space="PSUM") as ps_fin,
        ):
            id65 = cpool.tile([65, 65], F32)
            nc.sync.dma_start(out=id65[:], in_=id65_d[:])
            idhi = cpool.tile([128, 64], F16)  # identity on partitions 64:128
            nc.sync.dma_start(out=idhi[:], in_=idhi_d[:])

            for bh in range(NBH):
                qT_sb = bhpool.tile([D, S], F16, tag="qT")
                nc.sync.dma_start(out=qT_sb[:], in_=qT_d[bh])
                kvT_sb = bhpool.tile([128, S], F16, tag="kvT")
                nc.sync.dma_start(out=kvT_sb[:], in_=kvT_d[bh])

                # V stagings (natural layout), each with a ones-column
                vnat = bhpool.tile([128, 32, 65], F16, tag="vnat")
                nc.sync.dma_start(
                    out=vnat[:, :, 0:64],
                    in_=v_d[bh][:].rearrange("(m p) d -> p m d", p=128),
                )
                nc.vector.memset(vnat[:, :, 64:65], 1.0)
                vshift = bhpool.tile([128, 31, 65], F16, tag="vshift")
                nc.sync.dma_start(
                    out=vshift[:, :, 0:64],
                    in_=v_d[bh][64:4032, :].rearrange("(m p) d -> p m d", p=128),
                )
                nc.vector.memset(vshift[:, :, 64:65], 1.0)
                vg = bhpool.tile([128, 65], F16, tag="vg")
                nc.sync.dma_start(out=vg[:, 0:64], in_=_rows2(v_d[bh], 0, 4032, 64))
                nc.vector.memset(vg[:, 64:65], 1.0)
                vm0 = bhpool.tile([128, 65], F16, tag="vm0")
                nc.sync.dma_start(out=vm0[:, 0:64], in_=v_d[bh][0:128, :])
                nc.vector.memset(vm0[:, 64:65], 1.0)
                vm1 = bhpool.tile([128, 65], F16, tag="vm1")
                nc.sync.dma_start(out=vm1[:, 0:64], in_=_rows2(v_d[bh], 128, 4032, 64))
                nc.vector.memset(vm1[:, 64:65], 1.0)

                # batched K^T+V^T random/window-edge column gather on GPSIMD
                idxg_sb = bhpool.tile([128, NG // 16], I16, tag="idxg")
                nc.sync.dma_start(out=idxg_sb[:], in_=idxg_d[bh])
                ktr = bhpool.tile([128, NG * BS], F16, tag="ktr")
                # split the gather so early pairs start before the whole
                # (b,h)'s random blocks have landed
                half = NG // 2
                for hg in range(2):
                    nc.gpsimd.ap_gather(
                        out_ap=ktr[:, hg * half * BS:(hg + 1) * half * BS],
                        in_ap=kvT_sb[:],
                        idxs_ap=idxg_sb[:, hg * (half // 16):(hg + 1) * (half // 16)],
                        channels=128,
                        num_elems=NBLK,
                        d=BS,
                        num_idxs=half,
                    )

                # contiguous stagings for the global (0, 63) kT columns
                ktg = bhpool.tile([D, 128], F16, tag="ktg")
                nc.vector.tensor_copy(ktg[:, 0:64], kvT_sb[0:64, 0:64])
                nc.vector.tensor_copy(ktg[:, 64:128], kvT_sb[0:64, 4032:4096])
                ktg2 = bhpool.tile([D, 128], F16, tag="ktg2")  # blocks (2, 63)
                nc.vector.tensor_copy(ktg2[:, 0:64], kvT_sb[0:64, 128:192])
                nc.vector.tensor_copy(ktg2[:, 64:128], kvT_sb[0:64, 4032:4096])

                # batched middle-pair output staging (one DMA per (b,h))
                ctx_all = bhpool.tile([128, NMID, 64], F32, tag="ctx_all")

                def finish_pair(ctxT, idhalf, out_spec):
                    """ctxT psum [65,128] -> transpose -> normalize -> out."""
                    ctxT_sb = fpool.tile([65, 128], F32, tag="ctxT_sb")
                    nc.vector.tensor_copy(ctxT_sb[:], ctxT[:])
                    ctx2 = ps_fin.tile([128, 65], F32, tag="fin")
                    nc.tensor.transpose(ctx2[:], ctxT_sb[:], id65[:])
                    recip = fpool.tile([128, 1], F32, tag="recip")
                    nc.vector.reciprocal(recip[:], ctx2[:, 64:65])
                    if out_spec[0] == "mid":
                        # alternate the normalize between ACT and DVE to
                        # balance engine load (ACT also runs every exp)
                        if out_spec[1] % 2 == 0:
                            nc.scalar.activation(
                                out=ctx_all[:, out_spec[1], :], in_=ctx2[:, 0:64],
                                func=mybir.ActivationFunctionType.Copy,
                                scale=recip[:, 0:1],
                            )
                        else:
                            nc.vector.tensor_scalar_mul(
                                ctx_all[:, out_spec[1], :], ctx2[:, 0:64],
                                recip[:, 0:1],
                            )
                    else:
                        outsb = fpool.tile([128, 64], F32, tag="outsb")
                        nc.scalar.activation(
                            out=outsb[:], in_=ctx2[:, 0:64],
                            func=mybir.ActivationFunctionType.Copy,
                            scale=recip[:, 0:1],
                        )
                        nc.sync.dma_start(
                            out=_rows2(out_d[bh], out_spec[1], out_spec[2], 64),
                            in_=outsb[:],
                        )

                def make_vr(nslots, slot0, tag):
                    """PE-transpose gathered V^T slot pairs into natural-layout
                    V chunks [128, n, 65] with a ones column."""
                    n = nslots // 2
                    vtrT = ps_fin.tile([128, 4 * BS], F16, tag="fin")
                    for j in range(n):
                        base = BS * (slot0 + 2 * j)
                        nc.tensor.transpose(
                            vtrT[:, BS * j:BS * (j + 1)],
                            ktr[64:128, base:base + 128],
                            idhi[64:128, :],
                        )
                    vr = wpool.tile([128, 4, 65], F16, tag=tag)
                    nc.vector.tensor_copy(
                        vr[:, 0:n, 0:64],
                        vtrT[:, 0:n * BS].rearrange("p (a w) -> p a w", w=BS),
                    )
                    nc.vector.memset(vr[:, 0:n, 64:65], 1.0)
                    return vr

                # ---------------- full pair: rows 0 & 63 ----------------
                qts = wpool.tile([D, 128], F16, tag="qts")
                nc.vector.tensor_copy(qts[:, 0:64], qT_sb[:, 0:64])
                nc.vector.tensor_copy(qts[:, 64:128], qT_sb[:, 4032:4096])
                chunks = [(ktg[:], vg[:])]
                chunks += [
                    (kvT_sb[0:64, 64 + 128 * m:192 + 128 * m], vshift[:, m, :])
                    for m in range(31)
                ]
                ctxT = ps_ctxT.tile([65, 128], F32, tag="ctxT")
                ci = 0
                for g0 in range(0, 32, 4):
                    gch = chunks[g0:g0 + 4]
                    s_ps = ps_s.tile([128, 4, 128], F32, tag="s")
                    pt = wpool.tile([128, 4, 128], F16, tag="pt")
                    for j, (kap, _) in enumerate(gch):
                        nc.tensor.matmul(out=s_ps[:, j, :], lhsT=kap, rhs=qts[:],
                                         start=True, stop=True)
                    nc.scalar.activation(
                        out=pt[:], in_=s_ps[:],
                        func=mybir.ActivationFunctionType.Exp, scale=0.125)
                    for j, (_, vap) in enumerate(gch):
                        nc.tensor.matmul(out=ctxT[:], lhsT=vap, rhs=pt[:, j, :],
                                         start=(ci == 0), stop=(ci == 31),
                                         skip_group_check=True)
                        ci += 1
                finish_pair(ctxT, None, ("two", 0, 4032))

                # ---------------- special pair: rows 1 & 62 ----------------
                qts = wpool.tile([D, 128], F16, tag="qts")
                nc.vector.tensor_copy(qts[:, 0:64], qT_sb[:, 64:128])
                nc.vector.tensor_copy(qts[:, 64:128], qT_sb[:, 3968:4032])
                vr = make_vr(6, 0, "vr_sp")
                chunks = [
                    (kvT_sb[0:64, 0:128], vm0[:]),       # blocks (0, 1)
                    (ktg2[:], vm1[:]),                   # blocks (2, 63)
                    (kvT_sb[0:64, 3904:4032], vshift[:, 30, :]),  # (61, 62)
                    (ktr[0:64, 0:128], vr[:, 0, :]),     # (rA0, rA1)
                    (ktr[0:64, 128:256], vr[:, 1, :]),   # (rA2, rB0)
                    (ktr[0:64, 256:384], vr[:, 2, :]),   # (rB1, rB2)
                ]
                # zero invalid P^T entries (row62 must not see A's blocks etc)
                sp_masks = [
                    (slice(64, 128), 0, slice(64, 128)),  # blk1 not for row62
                    (slice(0, 64), 1, slice(64, 128)),    # blk2 not for row62
                    (slice(0, 128), 2, slice(0, 64)),     # 61,62 not for row1
                    (slice(0, 128), 3, slice(64, 128)),   # randA not for row62
                    (slice(0, 64), 0, slice(64, 128), 1),  # rA2 not for row62
                    (slice(64, 128), 0, slice(0, 64), 1),  # rB0 not for row1
                    (slice(0, 128), 1, slice(0, 64), 1),   # randB not for row1
                ]
                ctxT = ps_ctxT.tile([65, 128], F32, tag="ctxT")
                ci = 0
                for g0 in range(0, 6, 4):
                    gch = chunks[g0:g0 + 4]
                    gi = g0 // 4
                    s_ps = ps_s.tile([128, 4, 128], F32, tag="s")
                    pt = wpool.tile([128, 4, 128], F16, tag="pt")
                    for j, (kap, _) in enumerate(gch):
                        nc.tensor.matmul(out=s_ps[:, j, :], lhsT=kap, rhs=qts[:],
                                         start=True, stop=True)
                    nc.scalar.activation(
                        out=pt[:, 0:len(gch), :], in_=s_ps[:, 0:len(gch), :],
                        func=mybir.ActivationFunctionType.Exp, scale=0.125)
                    for m in sp_masks:
                        if (m[3] if len(m) > 3 else 0) == gi:
                            nc.vector.memset(pt[m[0], m[1], m[2]], 0.0)
                    for j, (_, vap) in enumerate(gch):
                        nc.tensor.matmul(out=ctxT[:], lhsT=vap, rhs=pt[:, j, :],
                                         start=(ci == 0), stop=(ci == 5),
                                         skip_group_check=True)
                        ci += 1
                finish_pair(ctxT, None, ("two", 64, 3968))

                # ---------------- middle pairs ----------------
                for l in range(NMID):
                    sl0 = 6 + 8 * l
                    q_ap = qT_sb[:, 128 * (l + 1):128 * (l + 2)]
                    qA = qT_sb[:, 128 * (l + 1):128 * (l + 1) + 64]
                    qB = qT_sb[:, 128 * (l + 1) + 64:128 * (l + 2)]
                    vr = make_vr(8, sl0, "vr")
                    s_ps = ps_s.tile([128, 4, 128], F32, tag="s")
                    pt = wpool.tile([128, 4, 128], F16, tag="pt")
                    # shared chunks: window middle (2l+2, 2l+3) and globals
                    nc.tensor.matmul(
                        out=s_ps[:, 0, :],
                        lhsT=kvT_sb[0:64, 128 * (l + 1):128 * (l + 2)],
                        rhs=q_ap, start=True, stop=True)
                    nc.tensor.matmul(
                        out=s_ps[:, 1, :], lhsT=ktg[:], rhs=q_ap,
                        start=True, stop=True)
                    # per-half chunks share PSUM columns: A in q-cols 0:64,
                    # B in 64:128
                    kb = BS * sl0
                    nc.tensor.matmul(out=s_ps[:, 2, 0:64],
                                     lhsT=ktr[0:64, kb:kb + 128],
                                     rhs=qA, start=True, stop=True)
                    nc.tensor.matmul(out=s_ps[:, 3, 0:64],
                                     lhsT=ktr[0:64, kb + 128:kb + 256],
                                     rhs=qA, start=True, stop=True)
                    nc.tensor.matmul(out=s_ps[:, 2, 64:128],
                                     lhsT=ktr[0:64, kb + 256:kb + 384],
                                     rhs=qB, start=True, stop=True)
                    nc.tensor.matmul(out=s_ps[:, 3, 64:128],
                                     lhsT=ktr[0:64, kb + 384:kb + 512],
                                     rhs=qB, start=True, stop=True)
                    nc.scalar.activation(
                        out=pt[:], in_=s_ps[:],
                        func=mybir.ActivationFunctionType.Exp, scale=0.125)
                    ctxT = ps_ctxT.tile([65, 128], F32, tag="ctxT")
                    nc.tensor.matmul(out=ctxT[:], lhsT=vnat[:, l + 1, :],
                                     rhs=pt[:, 0, :], start=True, stop=False,
                                     skip_group_check=True)
                    nc.tensor.matmul(out=ctxT[:], lhsT=vg[:],
                                     rhs=pt[:, 1, :], start=False, stop=False,
                                     skip_group_check=True)
                    nc.tensor.matmul(out=ctxT[:, 0:64], lhsT=vr[:, 0, :],
                                     rhs=pt[:, 2, 0:64], start=False, stop=False,
                                     skip_group_check=True)
                    nc.tensor.matmul(out=ctxT[:, 0:64], lhsT=vr[:, 1, :],
                                     rhs=pt[:, 3, 0:64], start=False, stop=False,
                                     skip_group_check=True)
                    nc.tensor.matmul(out=ctxT[:, 64:128], lhsT=vr[:, 2, :],
                                     rhs=pt[:, 2, 64:128], start=False, stop=False,
                                     skip_group_check=True)
                    nc.tensor.matmul(out=ctxT[:, 64:128], lhsT=vr[:, 3, :],
                                     rhs=pt[:, 3, 64:128], start=False, stop=True,
                                     skip_group_check=True)
                    finish_pair(ctxT, None, ("mid", l))

                # one batched output DMA for the 30 middle pairs
                nc.sync.dma_start(
                    out=out_d[bh][128:3968, :].rearrange(
                        "(m p) d -> p m d", p=128),
                    in_=ctx_all[:],
                )

    nc.compile()
    return nc


_PROGRAM = None


def _get_program():
    global _PROGRAM
    if _PROGRAM is None:
        _PROGRAM = build_program()
    return _PROGRAM


def make_core_inputs(q, k, v, rand, bh_slice):
    """Build one core's input map from full [32, S, D] arrays (fp32)."""
    qs = q[bh_slice]
    ks = k[bh_slice]
    vs = v[bh_slice]
    rs = rand[bh_slice]  # [NBH, 62, 3]
    qT = np.ascontiguousarray(qs.transpose(0, 2, 1)).astype(np.float16)
    kvT = np.concatenate(
        [qs.transpose(0, 2, 1) * 0, qs.transpose(0, 2, 1) * 0], axis=1
    ).astype(np.float16)
    kvT[:, 0:64, :] = ks.transpose(0, 2, 1).astype(np.float16)
    kvT[:, 64:128, :] = vs.transpose(0, 2, 1).astype(np.float16)
    vv = vs.astype(np.float16)

    # gather slot list per (b,h): 6 special + 8 per middle pair, pad to NG
    slots = np.zeros((NBH, NG), np.int16)
    for n in range(NBH):
        ra, rb = rs[n, 0], rs[n, 61]
        slots[n, 0:6] = np.concatenate([ra, rb])      # special: rA(3), rB(3)
        for l in range(NMID):
            sl0 = 6 + 8 * l
            ra, rb = rs[n, 2 * l + 1], rs[n, 2 * l + 2]
            slots[n, sl0] = 2 * l + 1                 # window edge A
            slots[n, sl0 + 1:sl0 + 4] = ra
            slots[n, sl0 + 4] = 2 * l + 4             # window edge B
            slots[n, sl0 + 5:sl0 + 8] = rb
    # ap_gather wrapped layout: index i -> partition 16g + i%16, col i//16
    idxg = np.empty((NBH, 128, NG // 16), np.int16)
    for n in range(NBH):
        wrapped = slots[n].reshape(NG // 16, 16).T    # [16, NG//16]
        idxg[n] = np.tile(wrapped, (8, 1))

    id65 = np.eye(65, dtype=np.float32)
    idhi = np.zeros((128, 64), np.float16)
    idhi[64:128, :] = np.eye(64, dtype=np.float16)
    return {"qT": qT, "kvT": kvT, "v": vv, "idxg": idxg,
            "id65": id65, "idhi": idhi}


def kernel(query, key, value, from_blocked_mask=None, to_blocked_mask=None,
           rand_attn=None, **_ignored):
    # masks are all-ones in this problem's input distribution; the block
    # structure (window/global/random) is handled explicitly.
    q = np.asarray(query, np.float32).reshape(B * H, S, D)
    k = np.asarray(key, np.float32).reshape(B * H, S, D)
    v = np.asarray(value, np.float32).reshape(B * H, S, D)
    rand = np.asarray(rand_attn).reshape(B * H, NBLK - 2, 3).astype(np.int32)

    in_maps = [
        make_core_inputs(q, k, v, rand, slice(c * NBH, (c + 1) * NBH))
        for c in range(NCORES)
    ]
    nc = _get_program()
    from concourse import bass_utils
    res = bass_utils.run_bass_kernel_spmd(nc, in_maps, core_ids=list(range(NCORES)))
    out = np.stack([r["out"] for r in res.results])  # [8, NBH, S, D]
    return out.reshape(B, H, S, D).astype(np.float32)


if __name__ == "__main__":
    nc = build_program()
    print("program built ok")
